# revision 2
# baseline (speedup 1.0000x reference)
"""Multi-head attention (B=4, T=2048, D=1024, H=16) on 8 Trainium2 NeuronCores.

Sharding: core c = (batch b = c//2, head-group g = c%2).  Each core computes
attention for one batch and 8 of the 16 heads (head-group = column slice of
WQ/WK/WV, row slice of WO), producing a partial output projection; the host
sums the two partials per batch and adds bO.

Per-core pipeline (all matmul inputs fp16, accumulation fp32 in PSUM):
  xT [D,T] -> qT,kT [512,T] (head dim on partitions) and v [T,512]
  S^T tile  = kT_slice.T @ qT_slab      [128 k, <=512 q]  (PSUM)
  E         = exp(S^T / 8)  fp16, causal-masked via 0/1 keep tiles
  ctx^T/den = [v | ones].T @ E          accumulated over k blocks (PSUM)
  ctx_norm  = ctx^T * (1/den)  (den reciprocal broadcast via DRAM round trip)
  out_part  = ctx^T pairs.T @ WO_slice  -> [T, 1024] fp32 partial

The mask input is analyzed on the host per 128x128 block (all-masked blocks
are skipped, partially masked blocks get a keep-tile multiply), so any mask
works; the causal mask collapses to one shared keep tile and ~2x compute
savings.
"""

import os
import sys
import numpy as np

B, T_FULL, D, H, DH = 4, 2048, 1024, 16, 64
P = 128
DL = 512          # per-core head-group width (8 heads x 64)
KD = D // P       # contraction chunks over D
N_CORES = 8

_CACHE = {}


def _import_concourse():
    for p in ("/opt/trn_rl_repo", "/root/.axon_site/_ro/trn_rl_repo"):
        if os.path.isdir(p) and p not in sys.path:
            sys.path.insert(0, p)
    import concourse.mybir as mybir  # noqa: F401
    import concourse.tile as tile  # noqa: F401
    from concourse import bacc  # noqa: F401
    from concourse import bass_utils  # noqa: F401
    return mybir, tile, bacc, bass_utils


def _mask_plan(maskT):
    """maskT: bool [T,T] in [k, q] orientation, True = masked out.

    Returns (plan, keep_tiles):
      plan[J] = list of blocks [I, c0, c1, subs]; subs = list of
      (col_off, cls, keep_idx) with cls in {'plain','mask','zero'}.
    """
    T = maskT.shape[0]
    TB, NS = T // P, T // 512
    keeps = {}
    keep_tiles = []
    plan = []
    for J in range(NS):
        blocks = []
        for I in range(TB):
            subs = []
            nz = []
            for s in range(4):
                qb = 4 * J + s
                blk = maskT[I * P:(I + 1) * P, qb * P:(qb + 1) * P]
                if blk.all():
                    subs.append((128 * s, "zero", None))
                elif blk.any():
                    kt = (~blk).astype(np.float16)
                    key = kt.tobytes()
                    if key not in keeps:
                        keeps[key] = len(keep_tiles)
                        keep_tiles.append(kt)
                    subs.append((128 * s, "mask", keeps[key]))
                    nz.append(s)
                else:
                    subs.append((128 * s, "plain", None))
                    nz.append(s)
            if not nz:
                continue
            blocks.append([I, 128 * min(nz), 128 * (max(nz) + 1), subs])
        if blocks:
            u0 = min(b[1] for b in blocks)
            u1 = max(b[2] for b in blocks)
            # first block covers the slab's union range so its start=True
            # matmul initializes every PSUM column the later blocks touch
            blocks[0][1], blocks[0][2] = u0, u1
        plan.append(blocks)
    return plan, keep_tiles


def _build(T, plan, n_keep):
    mybir, tile, bacc, _ = _import_concourse()
    F16, F32 = mybir.dt.float16, mybir.dt.float32
    Exp = mybir.ActivationFunctionType.Exp
    TB, NS = T // P, T // 512

    nc = bacc.Bacc("TRN2", target_bir_lowering=False, debug=False,
                   num_devices=N_CORES)
    xT = nc.dram_tensor("xT", [D, T], F16, kind="ExternalInput")
    wq = nc.dram_tensor("wq", [D, DL], F16, kind="ExternalInput")
    wk = nc.dram_tensor("wk", [D, DL], F16, kind="ExternalInput")
    wv = nc.dram_tensor("wv", [D, DL], F16, kind="ExternalInput")
    wo = nc.dram_tensor("wo", [DL, D], F16, kind="ExternalInput")
    keep = nc.dram_tensor("keep", [n_keep, P, P], F16, kind="ExternalInput")
    outp = nc.dram_tensor("outp", [T, D], F32, kind="ExternalOutput")

    with tile.TileContext(nc) as tc:
        with tc.tile_pool(name="persist", bufs=1) as persist:
            xT_sb = [persist.tile([P, T], F16, tag=f"xT{k}", name=f"xT{k}") for k in range(KD)]
            wq_sb = [persist.tile([P, DL], F16, tag=f"wq{k}", name=f"wq{k}") for k in range(KD)]
            wk_sb = [persist.tile([P, DL], F16, tag=f"wk{k}", name=f"wk{k}") for k in range(KD)]
            wv_sb = [persist.tile([P, DL], F16, tag=f"wv{k}", name=f"wv{k}") for k in range(KD)]
            wo_sb = [persist.tile([P, D], F16, tag=f"wo{k}", name=f"wo{k}") for k in range(4)]
            keep_sb = persist.tile([P, n_keep * P], F16, tag="keep", name="keep")
            qT_sb = [persist.tile([P, T], F16, tag=f"qT{i}", name=f"qT{i}") for i in range(4)]
            kT_sb = [persist.tile([P, T], F16, tag=f"kT{i}", name=f"kT{i}") for i in range(4)]
            va_sb = [persist.tile([P, 8 * 66], F16, tag=f"va{t}", name=f"va{t}") for t in range(TB)]
            cp_sb = [persist.tile([P, T], F16, tag=f"cp{i}", name=f"cp{i}") for i in range(4)]

            for k in range(KD):
                nc.sync.dma_start(xT_sb[k][:, :], xT[k * P:(k + 1) * P, :])
                nc.sync.dma_start(wq_sb[k][:, :], wq[k * P:(k + 1) * P, :])
                nc.sync.dma_start(wk_sb[k][:, :], wk[k * P:(k + 1) * P, :])
                nc.sync.dma_start(wv_sb[k][:, :], wv[k * P:(k + 1) * P, :])
            for k in range(4):
                nc.sync.dma_start(wo_sb[k][:, :], wo[k * P:(k + 1) * P, :])
            for i in range(n_keep):
                nc.sync.dma_start(keep_sb[:, i * P:(i + 1) * P], keep[i, :, :])

            # ---- phase 1: projections -------------------------------------
            with tc.tile_pool(name="pps", bufs=4, space="PSUM") as pps:
                for n in range(NS):
                    nsl = slice(n * 512, (n + 1) * 512)
                    for m in range(4):
                        for w_sb, t_sb in ((wq_sb, qT_sb), (wk_sb, kT_sb)):
                            ps = pps.tile([P, 512], F32, tag="ps", name="ps")
                            for k in range(KD):
                                nc.tensor.matmul(
                                    ps[:, :],
                                    w_sb[k][:, m * P:(m + 1) * P],
                                    xT_sb[k][:, nsl],
                                    start=(k == 0), stop=(k == KD - 1))
                            nc.any.tensor_copy(t_sb[m][:, nsl], ps[:, :])
                for t in range(TB):
                    ps = pps.tile([P, 512], F32, tag="ps", name="ps")
                    for k in range(KD):
                        nc.tensor.matmul(
                            ps[:, :],
                            xT_sb[k][:, t * P:(t + 1) * P],
                            wv_sb[k][:, :],
                            start=(k == 0), stop=(k == KD - 1))
                    va3 = va_sb[t].rearrange("p (h c) -> p h c", c=66)
                    ps3 = ps.rearrange("p (h c) -> p h c", c=64)
                    nc.vector.tensor_copy(va3[:, :, 0:64], ps3[:, :, :])
                    nc.any.memset(va3[:, :, 64:65], 1.0)

            # ---- phase 2: attention per head-pair -------------------------
            with tc.tile_pool(name="sps", bufs=4, space="PSUM") as sps_pool, \
                 tc.tile_pool(name="cps", bufs=4, space="PSUM") as cps_pool, \
                 tc.tile_pool(name="ework", bufs=8) as ework, \
                 tc.tile_pool(name="nwork", bufs=4) as nwork, \
                 tc.tile_pool(name="dsc", bufs=4, space="DRAM") as dscp:
                for hp in range(4):
                    for J in range(NS):
                        blocks = plan[J]
                        if not blocks:
                            continue
                        ctx_ps = [cps_pool.tile([P, 512], F32, tag="ctx", name="ctx")
                                  for _ in range(2)]
                        nb = len(blocks)
                        for bi, (I, c0, c1, subs) in enumerate(blocks):
                            for par in range(2):
                                h = 2 * hp + par
                                rows = slice(64 * par, 64 * par + 64)
                                sp = sps_pool.tile([P, 512], F32, tag="s", name="s")
                                nc.tensor.matmul(
                                    sp[:, c0:c1],
                                    kT_sb[hp][rows, I * P:(I + 1) * P],
                                    qT_sb[hp][rows, J * 512 + c0:J * 512 + c1],
                                    start=True, stop=True)
                                E = ework.tile([P, 512], F16, tag="E", name="E")
                                nc.scalar.activation(E[:, c0:c1], sp[:, c0:c1],
                                                     Exp, scale=0.125)
                                for cs, cls, idx in subs:
                                    if cs < c0 or cs >= c1:
                                        continue
                                    if cls == "mask":
                                        nc.vector.tensor_mul(
                                            E[:, cs:cs + P], E[:, cs:cs + P],
                                            keep_sb[:, idx * P:(idx + 1) * P])
                                    elif cls == "zero":
                                        nc.any.memset(E[:, cs:cs + P], 0.0)
                                nc.tensor.matmul(
                                    ctx_ps[par][0:65, c0:c1],
                                    va_sb[I][:, h * 66:h * 66 + 65],
                                    E[:, c0:c1],
                                    start=(bi == 0), stop=(bi == nb - 1),
                                    skip_group_check=True)
                        for par in range(2):
                            rec = nwork.tile([65, 512], F32, tag="rec", name="rec")
                            nc.vector.reciprocal(rec[64:65, :],
                                                 ctx_ps[par][64:65, :])
                            dsc = dscp.tile([1, 512], F32, tag="dsc", name="dsc")
                            nc.sync.dma_start(dsc[:, :], rec[64:65, :])
                            rb = nwork.tile([64, 512], F32, tag="rb", name="rb")
                            nc.sync.dma_start(
                                rb[:, :], dsc[0:1, :].to_broadcast((64, 512)))
                            jsl = slice(J * 512, (J + 1) * 512)
                            if par == 0:
                                nc.vector.tensor_mul(
                                    cp_sb[hp][0:64, jsl],
                                    ctx_ps[par][0:64, :], rb[:, :])
                            else:
                                tmp = nwork.tile([64, 512], F16, tag="tmp", name="tmp")
                                nc.vector.tensor_mul(
                                    tmp[:, :], ctx_ps[par][0:64, :], rb[:, :])
                                nc.sync.dma_start(cp_sb[hp][64:128, jsl],
                                                  tmp[:, :])

            # ---- phase 3: output projection (partial) ---------------------
            with tc.tile_pool(name="ops", bufs=4, space="PSUM") as ops_pool, \
                 tc.tile_pool(name="ost", bufs=4) as ost:
                for m in range(TB):
                    for n2 in range(2):
                        ps = ops_pool.tile([P, 512], F32, tag="o", name="o")
                        for kc in range(4):
                            nc.tensor.matmul(
                                ps[:, :],
                                cp_sb[kc][:, m * P:(m + 1) * P],
                                wo_sb[kc][:, n2 * 512:(n2 + 1) * 512],
                                start=(kc == 0), stop=(kc == 3))
                        ob = ost.tile([P, 512], F32, tag="ob", name="ob")
                        nc.any.tensor_copy(ob[:, :], ps[:, :])
                        nc.sync.dma_start(
                            outp[m * P:(m + 1) * P, n2 * 512:(n2 + 1) * 512],
                            ob[:, :])

    nc.compile()
    return nc


def _np_softmax_ref(x, mask, WQ, bQ, WK, bK, WV, bV, WO, bO):
    """Chunked numpy fallback, exact semantics of the reference."""
    Bc, Tc, Dc = x.shape
    q = (x @ WQ + bQ).reshape(Bc, Tc, H, DH).transpose(0, 2, 1, 3)
    k = (x @ WK + bK).reshape(Bc, Tc, H, DH).transpose(0, 2, 1, 3)
    v = (x @ WV + bV).reshape(Bc, Tc, H, DH).transpose(0, 2, 1, 3)
    m2 = mask[0, 0].astype(np.float32) * np.float32(-1e9)
    out = np.empty((Bc, Tc, Dc), np.float32)
    ctx = np.empty((Tc, H, DH), np.float32)
    for b in range(Bc):
        for h in range(H):
            s = (q[b, h] @ k[b, h].T) / np.float32(np.sqrt(DH)) + m2
            s -= s.max(axis=-1, keepdims=True)
            e = np.exp(s)
            ctx[:, h, :] = (e / e.sum(axis=-1, keepdims=True)) @ v[b, h]
        out[b] = ctx.reshape(Tc, Dc) @ WO + bO
    return out


def _get_compiled(T, plan, keep_tiles):
    key = (T, len(keep_tiles),
           tuple((J, tuple((b[0], b[1], b[2],
                            tuple((s[0], s[1], s[2]) for s in b[3]))
                           for b in blocks))
                 for J, blocks in enumerate(plan)))
    if key not in _CACHE:
        _CACHE[key] = _build(T, plan, max(1, len(keep_tiles)))
    return _CACHE[key]


_last_results = None


def kernel(x, mask, WQ, bQ, WK, bK, WV, bV, WO, bO):
    global _last_results
    x = np.asarray(x, dtype=np.float32)
    mask = np.asarray(mask).astype(bool)
    WQ, WK, WV, WO = (np.asarray(w, dtype=np.float32) for w in (WQ, WK, WV, WO))
    bQ, bK, bV, bO = (np.asarray(b_, dtype=np.float32) for b_ in (bQ, bK, bV, bO))
    T = x.shape[1]

    mask2d = mask[0, 0]
    degenerate_rows = bool(mask2d.all(axis=1).any())
    if (np.any(bQ) or np.any(bK) or np.any(bV) or degenerate_rows
            or T % 512 != 0 or x.shape[0] != B or x.shape[2] != D):
        out = _np_softmax_ref(x, mask, WQ, bQ, WK, bK, WV, bV, WO, np.zeros_like(bO))
        return (out + bO).astype(np.float32)

    maskT = np.ascontiguousarray(mask2d.T)
    plan, keep_tiles = _mask_plan(maskT)
    if len(keep_tiles) > 96:
        out = _np_softmax_ref(x, mask, WQ, bQ, WK, bK, WV, bV, WO, np.zeros_like(bO))
        return (out + bO).astype(np.float32)

    nc = _get_compiled(T, plan, keep_tiles)
    _, _, _, bass_utils = _import_concourse()

    n_keep = max(1, len(keep_tiles))
    keep_arr = np.ones((n_keep, P, P), np.float16)
    for i, kt in enumerate(keep_tiles):
        keep_arr[i] = kt

    in_maps = []
    for c in range(N_CORES):
        b, g = c // 2, c % 2
        gsl = slice(g * DL, (g + 1) * DL)
        in_maps.append({
            "xT": np.ascontiguousarray(x[b].T).astype(np.float16),
            "wq": WQ[:, gsl].astype(np.float16),
            "wk": WK[:, gsl].astype(np.float16),
            "wv": WV[:, gsl].astype(np.float16),
            "wo": np.ascontiguousarray(WO[gsl, :]).astype(np.float16),
            "keep": keep_arr,
        })

    trace = os.environ.get("MHA_TRACE") == "1"
    kw = {}
    if trace:
        kw["trace"] = True
        tmpdir = os.environ.get("MHA_TMPDIR")
        if tmpdir:
            kw["tmpdir"] = tmpdir
    res = bass_utils.run_bass_kernel_spmd(nc, in_maps,
                                          core_ids=list(range(N_CORES)), **kw)
    _last_results = res

    out = np.empty((B, T, D), np.float32)
    for b in range(B):
        out[b] = res.results[2 * b]["outp"] + res.results[2 * b + 1]["outp"] + bO
    return out


# revision 5
# speedup vs baseline: 1.4853x; 1.4853x over previous
"""Multi-head attention (B=4, T=2048, D=1024, H=16) on 8 Trainium2 NeuronCores.

Sharding: core c = (batch b = c//2, head-group g = c%2).  Each core computes
attention for one batch and 8 of the 16 heads (head-group = column slice of
WQ/WK/WV, row slice of WO), producing a partial output projection; the host
sums the two partials per batch and adds bO.

Per-core pipeline (all matmul inputs fp16, accumulation fp32 in PSUM):
  xT [D,T] -> qT,kT [512,T] (head dim on partitions) and v [T,512]
  S^T tile  = kT_slice.T @ qT_slab      [128 k, <=512 q]  (PSUM)
  E         = exp(S^T / 8)  fp16, causal-masked via 0/1 keep tiles
  ctx^T/den = [v | ones].T @ E          accumulated over k blocks (PSUM)
  ctx_norm  = ctx^T * (1/den)  (den reciprocal broadcast via DRAM round trip)
  out_part  = ctx^T pairs.T @ WO_slice  -> [T, 1024] fp32 partial

The mask input is analyzed on the host per 128x128 block (all-masked blocks
are skipped, partially masked blocks get a keep-tile multiply), so any mask
works; the causal mask collapses to one shared keep tile and ~2x compute
savings.
"""

import os
import sys
import numpy as np

B, T_FULL, D, H, DH = 4, 2048, 1024, 16, 64
P = 128
DL = 512          # per-core head-group width (8 heads x 64)
KD = D // P       # contraction chunks over D
N_CORES = 8

_CACHE = {}


def _import_concourse():
    for p in ("/opt/trn_rl_repo", "/root/.axon_site/_ro/trn_rl_repo"):
        if os.path.isdir(p) and p not in sys.path:
            sys.path.insert(0, p)
    import concourse.mybir as mybir  # noqa: F401
    import concourse.tile as tile  # noqa: F401
    from concourse import bacc  # noqa: F401
    from concourse import bass_utils  # noqa: F401
    return mybir, tile, bacc, bass_utils


def _mask_plan(maskT):
    """maskT: bool [T,T] in [k, q] orientation, True = masked out.

    Returns (plan, keep_tiles):
      plan[J] = list of blocks [I, c0, c1, subs]; subs = list of
      (col_off, cls, keep_idx) with cls in {'plain','mask','zero'}.
    """
    T = maskT.shape[0]
    TB, NS = T // P, T // 512
    keeps = {}
    keep_tiles = []
    plan = []
    for J in range(NS):
        blocks = []
        for I in range(TB):
            subs = []
            nz = []
            for s in range(4):
                qb = 4 * J + s
                blk = maskT[I * P:(I + 1) * P, qb * P:(qb + 1) * P]
                if blk.all():
                    subs.append((128 * s, "zero", None))
                elif blk.any():
                    kt = (~blk).astype(np.float16)
                    key = kt.tobytes()
                    if key not in keeps:
                        keeps[key] = len(keep_tiles)
                        keep_tiles.append(kt)
                    subs.append((128 * s, "mask", keeps[key]))
                    nz.append(s)
                else:
                    subs.append((128 * s, "plain", None))
                    nz.append(s)
            if not nz:
                continue
            blocks.append([I, 128 * min(nz), 128 * (max(nz) + 1), subs])
        if blocks:
            u0 = min(b[1] for b in blocks)
            u1 = max(b[2] for b in blocks)
            # first block covers the slab's union range so its start=True
            # matmul initializes every PSUM column the later blocks touch
            blocks[0][1], blocks[0][2] = u0, u1
        plan.append(blocks)
    return plan, keep_tiles


def _build(T, plan, n_keep):
    mybir, tile, bacc, _ = _import_concourse()
    F16, F32 = mybir.dt.float16, mybir.dt.float32
    Exp = mybir.ActivationFunctionType.Exp
    TB, NS = T // P, T // 512

    nc = bacc.Bacc("TRN2", target_bir_lowering=False, debug=False,
                   num_devices=N_CORES)
    xT = nc.dram_tensor("xT", [D, T], F16, kind="ExternalInput")
    wq = nc.dram_tensor("wq", [D, DL], F16, kind="ExternalInput")
    wk = nc.dram_tensor("wk", [D, DL], F16, kind="ExternalInput")
    wv = nc.dram_tensor("wv", [D, DL], F16, kind="ExternalInput")
    wo = nc.dram_tensor("wo", [DL, D], F16, kind="ExternalInput")
    keep = nc.dram_tensor("keep", [n_keep, P, P], F16, kind="ExternalInput")
    outp = nc.dram_tensor("outp", [T, D], F32, kind="ExternalOutput")

    with tile.TileContext(nc) as tc:
        with tc.tile_pool(name="persist", bufs=1) as persist:
            xT_sb = [persist.tile([P, T], F16, tag=f"xT{k}", name=f"xT{k}") for k in range(KD)]
            wq_sb = [persist.tile([P, DL], F16, tag=f"wq{k}", name=f"wq{k}") for k in range(KD)]
            wk_sb = [persist.tile([P, DL], F16, tag=f"wk{k}", name=f"wk{k}") for k in range(KD)]
            wv_sb = [persist.tile([P, DL], F16, tag=f"wv{k}", name=f"wv{k}") for k in range(KD)]
            wo_sb = [persist.tile([P, D], F16, tag=f"wo{k}", name=f"wo{k}") for k in range(4)]
            keep_sb = persist.tile([P, n_keep * P], F16, tag="keep", name="keep")
            qT_sb = [persist.tile([P, T], F16, tag=f"qT{i}", name=f"qT{i}") for i in range(4)]
            kT_sb = [persist.tile([P, T], F16, tag=f"kT{i}", name=f"kT{i}") for i in range(4)]
            va_sb = [persist.tile([P, 8 * 66], F16, tag=f"va{t}", name=f"va{t}") for t in range(TB)]
            cp_sb = [persist.tile([P, T], F16, tag=f"cp{i}", name=f"cp{i}") for i in range(4)]

            warm_sink = persist.tile([P, 1], F32, tag="warmsink", name="warmsink")
            for k in range(KD):
                nc.sync.dma_start(wq_sb[k][:, :], wq[k * P:(k + 1) * P, :])
                nc.sync.dma_start(wk_sb[k][:, :], wk[k * P:(k + 1) * P, :])
                nc.sync.dma_start(wv_sb[k][:, :], wv[k * P:(k + 1) * P, :])
            for k in range(KD):
                nc.sync.dma_start(xT_sb[k][:, :], xT[k * P:(k + 1) * P, :])
            for k in range(4):
                nc.sync.dma_start(wo_sb[k][:, :], wo[k * P:(k + 1) * P, :])
            for i in range(n_keep):
                nc.sync.dma_start(keep_sb[:, i * P:(i + 1) * P], keep[i, :, :])

            # ---- phase 1: projections -------------------------------------
            with tc.tile_pool(name="pps", bufs=4, space="PSUM") as pps:
                # dummy matmuls on the first-arrived weight chunk keep the PE
                # active monitor warm while the x DMAs are still in flight
                warm = pps.tile([P, 512], F32, tag="warm", name="warm", bufs=1)
                for r in range(12):
                    nc.tensor.matmul(warm[:, :], wq_sb[0][:, 0:P], wq_sb[0][:, :],
                                     start=(r == 0), stop=(r == 11),
                                     skip_group_check=True)
                nc.any.tensor_copy(warm_sink[:, :], warm[:, 0:1])
                for n in range(NS):
                    nsl = slice(n * 512, (n + 1) * 512)
                    for m in range(4):
                        for w_sb, t_sb in ((wq_sb, qT_sb), (wk_sb, kT_sb)):
                            ps = pps.tile([P, 512], F32, tag="ps", name="ps")
                            for k in range(KD):
                                nc.tensor.matmul(
                                    ps[:, :],
                                    w_sb[k][:, m * P:(m + 1) * P],
                                    xT_sb[k][:, nsl],
                                    start=(k == 0), stop=(k == KD - 1))
                            nc.any.tensor_copy(t_sb[m][:, nsl], ps[:, :])
                for t in range(TB):
                    ps = pps.tile([P, 512], F32, tag="ps", name="ps")
                    for k in range(KD):
                        nc.tensor.matmul(
                            ps[:, :],
                            xT_sb[k][:, t * P:(t + 1) * P],
                            wv_sb[k][:, :],
                            start=(k == 0), stop=(k == KD - 1))
                    va3 = va_sb[t].rearrange("p (h c) -> p h c", c=66)
                    ps3 = ps.rearrange("p (h c) -> p h c", c=64)
                    nc.vector.tensor_copy(va3[:, :, 0:64], ps3[:, :, :])
                    nc.any.memset(va3[:, :, 64:65], 1.0)

            # ---- phase 2: attention per head-pair -------------------------
            # s/E tiles hold both head parities side by side (2 PSUM banks);
            # one exp ACTIVATE covers the pair, halving ACT op overhead.
            with tc.tile_pool(name="sps", bufs=2, space="PSUM") as sps_pool, \
                 tc.tile_pool(name="cps", bufs=4, space="PSUM") as cps_pool, \
                 tc.tile_pool(name="ework", bufs=8) as ework, \
                 tc.tile_pool(name="nwork", bufs=4) as nwork, \
                 tc.tile_pool(name="dsc", bufs=4, space="DRAM") as dscp:
                for hp in range(4):
                    for J in range(NS):
                        blocks = plan[J]
                        if not blocks:
                            continue
                        ctx_ps = [cps_pool.tile([P, 512], F32, tag="ctx", name="ctx")
                                  for _ in range(2)]
                        nb = len(blocks)
                        for bi, (I, c0, c1, subs) in enumerate(blocks):
                            sp = sps_pool.tile([P, 1024], F32, tag="s", name="s")
                            E = ework.tile([P, 1024], F16, tag="E", name="E")
                            for par in range(2):
                                rows = slice(64 * par, 64 * par + 64)
                                nc.tensor.matmul(
                                    sp[:, 512 * par + c0:512 * par + c1],
                                    kT_sb[hp][rows, I * P:(I + 1) * P],
                                    qT_sb[hp][rows, J * 512 + c0:J * 512 + c1],
                                    start=True, stop=True)
                            sp3 = sp.rearrange("p (a c) -> p a c", a=2)
                            E3 = E.rearrange("p (a c) -> p a c", a=2)
                            nc.scalar.activation(E3[:, :, c0:c1], sp3[:, :, c0:c1],
                                                 Exp, scale=0.125)
                            for par in range(2):
                                for cs, cls, idx in subs:
                                    if cs < c0 or cs >= c1:
                                        continue
                                    if cls == "mask":
                                        nc.vector.tensor_mul(
                                            E[:, 512 * par + cs:512 * par + cs + P],
                                            E[:, 512 * par + cs:512 * par + cs + P],
                                            keep_sb[:, idx * P:(idx + 1) * P])
                                    elif cls == "zero":
                                        nc.any.memset(
                                            E[:, 512 * par + cs:512 * par + cs + P],
                                            0.0)
                            for par in range(2):
                                h = 2 * hp + par
                                nc.tensor.matmul(
                                    ctx_ps[par][0:65, c0:c1],
                                    va_sb[I][:, h * 66:h * 66 + 65],
                                    E[:, 512 * par + c0:512 * par + c1],
                                    start=(bi == 0), stop=(bi == nb - 1),
                                    skip_group_check=True)
                        for par in range(2):
                            # evict to SBUF promptly so the PSUM bank frees
                            # without waiting for the normalization chain
                            ctxs = nwork.tile([65, 512], F32, tag="ctxs",
                                              name="ctxs")
                            nc.any.tensor_copy(ctxs[:, :], ctx_ps[par][0:65, :])
                            dsc = dscp.tile([1, 512], F32, tag="dsc", name="dsc")
                            nc.sync.dma_start(dsc[:, :], ctxs[64:65, :])
                            db = nwork.tile([64, 512], F32, tag="db", name="db")
                            nc.sync.dma_start(
                                db[:, :], dsc[0:1, :].to_broadcast((64, 512)))
                            # approx_fast needs base partition 0, so take the
                            # reciprocal on the broadcast denominator
                            rb = nwork.tile([64, 512], F32, tag="rb", name="rb")
                            nc.vector.reciprocal_approx_fast(rb[:, :], db[:, :])
                            jsl = slice(J * 512, (J + 1) * 512)
                            if par == 0:
                                nc.vector.tensor_mul(
                                    cp_sb[hp][0:64, jsl],
                                    ctxs[0:64, :], rb[:, :])
                            else:
                                tmp = nwork.tile([64, 512], F16, tag="tmp", name="tmp")
                                nc.vector.tensor_mul(
                                    tmp[:, :], ctxs[0:64, :], rb[:, :])
                                nc.sync.dma_start(cp_sb[hp][64:128, jsl],
                                                  tmp[:, :])

            # ---- phase 3: output projection (partial) ---------------------
            with tc.tile_pool(name="ops", bufs=4, space="PSUM") as ops_pool, \
                 tc.tile_pool(name="ost", bufs=4) as ost:
                for m in range(TB):
                    for n2 in range(2):
                        ps = ops_pool.tile([P, 512], F32, tag="o", name="o")
                        for kc in range(4):
                            nc.tensor.matmul(
                                ps[:, :],
                                cp_sb[kc][:, m * P:(m + 1) * P],
                                wo_sb[kc][:, n2 * 512:(n2 + 1) * 512],
                                start=(kc == 0), stop=(kc == 3))
                        ob = ost.tile([P, 512], F32, tag="ob", name="ob")
                        nc.any.tensor_copy(ob[:, :], ps[:, :])
                        nc.sync.dma_start(
                            outp[m * P:(m + 1) * P, n2 * 512:(n2 + 1) * 512],
                            ob[:, :])

    nc.compile()
    return nc


def _np_softmax_ref(x, mask, WQ, bQ, WK, bK, WV, bV, WO, bO):
    """Chunked numpy fallback, exact semantics of the reference."""
    Bc, Tc, Dc = x.shape
    q = (x @ WQ + bQ).reshape(Bc, Tc, H, DH).transpose(0, 2, 1, 3)
    k = (x @ WK + bK).reshape(Bc, Tc, H, DH).transpose(0, 2, 1, 3)
    v = (x @ WV + bV).reshape(Bc, Tc, H, DH).transpose(0, 2, 1, 3)
    m2 = mask[0, 0].astype(np.float32) * np.float32(-1e9)
    out = np.empty((Bc, Tc, Dc), np.float32)
    ctx = np.empty((Tc, H, DH), np.float32)
    for b in range(Bc):
        for h in range(H):
            s = (q[b, h] @ k[b, h].T) / np.float32(np.sqrt(DH)) + m2
            s -= s.max(axis=-1, keepdims=True)
            e = np.exp(s)
            ctx[:, h, :] = (e / e.sum(axis=-1, keepdims=True)) @ v[b, h]
        out[b] = ctx.reshape(Tc, Dc) @ WO + bO
    return out


def _get_compiled(T, plan, keep_tiles):
    key = (T, len(keep_tiles),
           tuple((J, tuple((b[0], b[1], b[2],
                            tuple((s[0], s[1], s[2]) for s in b[3]))
                           for b in blocks))
                 for J, blocks in enumerate(plan)))
    if key not in _CACHE:
        _CACHE[key] = _build(T, plan, max(1, len(keep_tiles)))
    return _CACHE[key]


_last_results = None


def kernel(x, mask, WQ, bQ, WK, bK, WV, bV, WO, bO):
    global _last_results
    x = np.asarray(x, dtype=np.float32)
    mask = np.asarray(mask).astype(bool)
    WQ, WK, WV, WO = (np.asarray(w, dtype=np.float32) for w in (WQ, WK, WV, WO))
    bQ, bK, bV, bO = (np.asarray(b_, dtype=np.float32) for b_ in (bQ, bK, bV, bO))
    T = x.shape[1]

    mask2d = mask[0, 0]
    degenerate_rows = bool(mask2d.all(axis=1).any())
    if (np.any(bQ) or np.any(bK) or np.any(bV) or degenerate_rows
            or T % 512 != 0 or x.shape[0] != B or x.shape[2] != D):
        out = _np_softmax_ref(x, mask, WQ, bQ, WK, bK, WV, bV, WO, np.zeros_like(bO))
        return (out + bO).astype(np.float32)

    maskT = np.ascontiguousarray(mask2d.T)
    plan, keep_tiles = _mask_plan(maskT)
    if len(keep_tiles) > 96:
        out = _np_softmax_ref(x, mask, WQ, bQ, WK, bK, WV, bV, WO, np.zeros_like(bO))
        return (out + bO).astype(np.float32)

    nc = _get_compiled(T, plan, keep_tiles)
    _, _, _, bass_utils = _import_concourse()

    n_keep = max(1, len(keep_tiles))
    keep_arr = np.ones((n_keep, P, P), np.float16)
    for i, kt in enumerate(keep_tiles):
        keep_arr[i] = kt

    in_maps = []
    for c in range(N_CORES):
        b, g = c // 2, c % 2
        gsl = slice(g * DL, (g + 1) * DL)
        in_maps.append({
            "xT": np.ascontiguousarray(x[b].T).astype(np.float16),
            "wq": WQ[:, gsl].astype(np.float16),
            "wk": WK[:, gsl].astype(np.float16),
            "wv": WV[:, gsl].astype(np.float16),
            "wo": np.ascontiguousarray(WO[gsl, :]).astype(np.float16),
            "keep": keep_arr,
        })

    trace = os.environ.get("MHA_TRACE") == "1"
    kw = {}
    if trace:
        kw["trace"] = True
        tmpdir = os.environ.get("MHA_TMPDIR")
        if tmpdir:
            kw["tmpdir"] = tmpdir
    res = bass_utils.run_bass_kernel_spmd(nc, in_maps,
                                          core_ids=list(range(N_CORES)), **kw)
    _last_results = res

    out = np.empty((B, T, D), np.float32)
    for b in range(B):
        out[b] = res.results[2 * b]["outp"] + res.results[2 * b + 1]["outp"] + bO
    return out
